# revision 25
# baseline (speedup 1.0000x reference)
"""Distributed 3-layer GAT on 8 TRN2 NeuronCores (Bass/Tile).

Sharding: core c owns dst nodes [c*NS, (c+1)*NS). Non-self-loop edges
partitioned by dst shard, sorted by (src-quarter, dst-chunk, dst);
quarter-major tile numbering. Weights replicated.

Gather table rows are 256 B: 120 bf16 features + 4 bf16 s_src scores +
8 fp8e4m3 features (wext columns permuted host-side so the dense matmul
emits them contiguously; error budget verified offline: rel ~1.4e-2).
s_dst lives in a separate tiny [NS,4] bf16 table consumed by the eqT
matmuls. Halving the row (512->256 B) halves both the SWDGE gather
stream and the AllGather volume.

Per layer: sharded dense (bf16) -> 4 chunked AllGathers -> edge phase:
dma_gather of src rows round-robined over SWDGE queues 1-3 (sc3 issued
one pair late so the AG3 wait never head-of-line blocks the gpsimd
queue), merged [eqT|eq] fp8 one-hot loads spread over gpsimd-q0 /
scalar / sync DMA queues, PE matmuls accumulate weighted messages +
softmax denominators in PSUM. Self-loops skip the gather entirely:
local rows + constant diagonal fp8 lhsT. alpha = ss + (eqT @ sd) in
f32, leaky-relu via Act Prelu, exp lands in mm-rhs cols 128:132 and
doubles as the broadcast weighting multiplier. Reciprocals batched per
pair.
"""
import os, sys, types
sys.path.insert(0, "/opt/trn_rl_repo")
import numpy as np
import ml_dtypes

import antenv
if "antenv.axon_hooks" not in sys.modules:
    _hooks_mod = types.ModuleType("antenv.axon_hooks")
    _HOOK = [None]
    _hooks_mod.set_axon_ntff_profile_hook = lambda h: _HOOK.__setitem__(0, h)
    _hooks_mod.get_axon_ntff_profile_hook = lambda: _HOOK[0]
    sys.modules["antenv.axon_hooks"] = _hooks_mod
    antenv.axon_hooks = _hooks_mod
    try:
        import trn_agent_boot.trn_boot as _tb
        _h = _tb._ntff_profile_via_ctypes("/opt/axon/libaxon_pjrt.so")
        if _h is not None:
            _hooks_mod.set_axon_ntff_profile_hook(_h)
    except Exception:
        pass

from concourse import bass, bacc, mybir, tile
from concourse import bass_utils
from concourse.bass_utils import run_bass_kernel_spmd
from concourse._compat import get_trn_type
bass_utils.upload_artifacts = lambda tmpdir: "local://noop"

F32 = mybir.dt.float32
BF16 = mybir.dt.bfloat16
FP8 = mybir.dt.float8e4
I16 = mybir.dt.int16
NPBF = ml_dtypes.bfloat16
NPF8 = ml_dtypes.float8_e4m3fn
P = 128
SLOPE = 0.2
AF = mybir.ActivationFunctionType

N = 100000
IN, HID, HEADS, OUT = 128, 32, 4, 32
NCORES = 8
NS = N // NCORES
NCH = (NS + P - 1) // P
PAIR_CH = 5
NSC = 4
ROWB = 128   # table row: 128 bf16 slots = 256 B
FEAT = 128
BFN = 120    # features stored in bf16 (row cols 0:120)
NCOL = 136   # dense output cols: 120 bf | 4 ws | 8 f8-feat | 4 wd
MMC = 132    # acc matmul rhs cols: 128 weighted feats + 4 exp

# quarter-major table layout: quarter q = per-core chunks [QCH[q], QCH[q+1])
QCH = [0, 30, 60, 90, 98]
S_ROW = [0, 3840, 7680, 11520]          # per-core row offset of quarter q
SZ = [3840, 3840, 3840, 980]            # per-core rows in quarter q
B_ROW = [0, 30720, 61440, 92160]        # global block start of quarter q
# AG for quarter q fires after the dense tile of chunk QCH[q+1]-1


def _build_wext(W, a_s, a_d):
    Fin = W.shape[0]
    H, C = a_s.shape
    Wr = W.reshape(Fin, H, C)
    ws = np.einsum("fhc,hc->fh", Wr, a_s)
    wd = np.einsum("fhc,hc->fh", Wr, a_d)
    # permuted: [feat0:120 | ws | feat120:128 | wd]
    return np.ascontiguousarray(
        np.concatenate([W[:, 0:BFN], ws, W[:, BFN:FEAT], wd], axis=1), np.float32)


def _perm_rows(v):
    """global node id -> permuted (quarter-major) table row"""
    c = v // NS
    l = v % NS
    q = np.searchsorted(S_ROW, l, side="right") - 1
    szq = np.asarray(SZ, np.int64)[q]
    return (np.asarray(B_ROW, np.int64)[q] + c * szq
            + (l - np.asarray(S_ROW, np.int64)[q])), q


def _prep_graph(edge_index):
    # self-loops are added separately (local rows, no gather); all original
    # edges stay (PyG adds loops on top of whatever is in edge_index).
    src = edge_index[0].astype(np.int64)
    dst = edge_index[1].astype(np.int64)
    prow_all, q_all = _perm_rows(src)

    core = dst // NS
    per_core = []
    counts = np.zeros((NCORES, NCH, NSC), dtype=np.int64)
    for c in range(NCORES):
        m = core == c
        s_c, d_c = prow_all[m], dst[m]
        sc = q_all[m]
        dl = d_c - c * NS
        ch = dl // P
        order = np.lexsort((d_c, ch, sc))
        s_c, dl, ch, sc = s_c[order], dl[order], ch[order], sc[order]
        np.add.at(counts[c], (ch, sc), 1)
        per_core.append((s_c, dl, ch, sc))

    T = np.ceil(counts.max(axis=0) / P).astype(np.int64)
    tile_off = np.zeros((NCH, NSC), dtype=np.int64)
    acc = 0
    for sc in range(NSC):
        for ch in range(NCH):
            tile_off[ch, sc] = acc
            acc += T[ch, sc]
    ntiles = acc

    gidx = np.zeros((NCORES, ntiles * P), dtype=np.int16)
    dstloc = np.full((NCORES, ntiles, P), -1.0, dtype=np.float32)
    for c in range(NCORES):
        s_c, dl, ch, sc = per_core[c]
        pos = 0
        for scv in range(NSC):
            for chv in range(NCH):
                n = int(counts[c, chv, scv])
                if n == 0:
                    continue
                t0 = int(tile_off[chv, scv])
                gidx[c, t0 * P: t0 * P + n] = (s_c[pos:pos + n] - B_ROW[scv]).astype(np.int16)
                dstloc[c].reshape(-1)[t0 * P: t0 * P + n] = (dl[pos:pos + n] - chv * P).astype(np.float32)
                pos += n
        assert pos == len(s_c)
    return T, tile_off, int(ntiles), gidx, dstloc


def _wrap_idx(flat16):
    n = flat16.shape[0]
    w = flat16.reshape(n // 16, 16).T
    return np.ascontiguousarray(np.tile(w, (8, 1)), np.int16)


def _build_program(T, tile_off, ntiles):
    nc = bacc.Bacc(get_trn_type() or "TRN2", target_bir_lowering=False,
                   debug=False, enable_asserts=False, num_devices=NCORES,
                   num_swdge_queues=4)
    x_sh = nc.dram_tensor("x_shard", [NS, FEAT], BF16, kind="ExternalInput").ap()
    gidx_t = nc.dram_tensor("gidx", [P, ntiles * 8], I16, kind="ExternalInput").ap()
    eqc_t = nc.dram_tensor("eqc", [P, ntiles * 2 * P], FP8, kind="ExternalInput").ap()
    wext_t = [nc.dram_tensor(f"wext{l}", [FEAT, NCOL], BF16, kind="ExternalInput").ap() for l in range(3)]
    btile_t = [nc.dram_tensor(f"btile{l}", [P, FEAT if l < 2 else OUT], F32, kind="ExternalInput").ap() for l in range(3)]
    ident_t = nc.dram_tensor("ident", [P, P], BF16, kind="ExternalInput").ap()
    diag_t = nc.dram_tensor("diag", [P, P], FP8, kind="ExternalInput").ap()
    out_t = nc.dram_tensor("out", [NS, OUT], F32, kind="ExternalOutput").ap()

    groups = [list(range(NCORES))]
    NPAIR = (NCH + PAIR_CH - 1) // PAIR_CH

    with tile.TileContext(nc) as tc:
        with (
            tc.tile_pool(name="const", bufs=1) as constp,
            tc.tile_pool(name="sched", bufs=1) as schedp,
            tc.tile_pool(name="gpool", bufs=9) as gpool,
            tc.tile_pool(name="eqp", bufs=8) as eqp,
            tc.tile_pool(name="work", bufs=5) as work,
            tc.tile_pool(name="wrhsp", bufs=2) as wrhsp,
            tc.tile_pool(name="nodep", bufs=4) as nodep,
            tc.tile_pool(name="dnmp", bufs=2) as dnmp,
            tc.tile_pool(name="accp", bufs=5, space="PSUM") as accp,
            tc.tile_pool(name="tpp", bufs=1, space="PSUM") as tpp,
            tc.tile_pool(name="sdp", bufs=2, space="PSUM") as sdp,
            tc.tile_pool(name="dram", bufs=2, space="DRAM") as dramp,
        ):
            ident = constp.tile([P, P], BF16, tag="ident")
            nc.sync.dma_start(out=ident[:], in_=ident_t[:])
            diag = constp.tile([P, P], FP8, tag="diag")
            nc.sync.dma_start(out=diag[:], in_=diag_t[:])
            wext, btile = [], []
            for l in range(3):
                w = constp.tile([FEAT, NCOL], BF16, tag=f"wext{l}")
                nc.sync.dma_start(out=w[:], in_=wext_t[l][:])
                wext.append(w)
                b = constp.tile([P, FEAT if l < 2 else OUT], F32, tag=f"bt{l}")
                nc.sync.dma_start(out=b[:], in_=btile_t[l][:])
                btile.append(b)
            gidx_sb = schedp.tile([P, ntiles * 8], I16, tag="gidx")
            nc.scalar.dma_start(out=gidx_sb[:], in_=gidx_t[:])

            xwss_sh = [dramp.tile([NS, ROWB], BF16, tag="xwsh", name=f"xwsh{i}") for i in range(3)]
            sdcache = schedp.tile([P, NCH, 4], BF16, tag="sdc")
            slcache = schedp.tile([P, NCH, ROWB], BF16, tag="slc")
            xwss_full = [[dramp.tile([NCORES * SZ[q], ROWB], BF16, tag=f"xwfull{q}",
                                     name=f"xwfull{i}_{q}", addr_space="Shared")
                          for q in range(NSC)] for i in range(3)]

            def fire_ag(layer, q):
                nc.gpsimd.collective_compute(
                    "AllGather", mybir.AluOpType.bypass, replica_groups=groups,
                    ins=[xwss_sh[layer][S_ROW[q]:S_ROW[q] + SZ[q], :].opt()],
                    outs=[xwss_full[layer][q][:].opt()])

            def dense_tile(h_sb, lnext, ch, base, nn, startup=False):
                hT_ps = tpp.tile([P, P], BF16, tag="tp")
                nc.tensor.transpose(out=hT_ps[:], in_=h_sb[:], identity=ident[:])
                hT_sb = work.tile([P, P], BF16, tag="hT")
                nc.scalar.copy(out=hT_sb[:], in_=hT_ps[:])
                if startup:
                    d_ps = accp.tile([P, NCOL], F32, tag="acc", name="d_ps_s")
                else:
                    d_ps = tpp.tile([P, NCOL], F32, tag="tp", name="d_ps")
                nc.tensor.matmul(out=d_ps[:], lhsT=hT_sb[:], rhs=wext[lnext][:],
                                 start=True, stop=True)
                nc.scalar.copy(out=slcache[:, ch, 0:BFN + 4], in_=d_ps[:, 0:BFN + 4])
                nc.scalar.copy(out=slcache[:, ch, BFN + 4:ROWB].bitcast(FP8),
                               in_=d_ps[:, BFN + 4:BFN + 12])
                nc.scalar.copy(out=sdcache[:, ch, :], in_=d_ps[:, BFN + 12:NCOL])
                nc.scalar.dma_start(out=xwss_sh[lnext][base:base + nn, :],
                                    in_=slcache[:nn, ch, :])

            ag_after_chunk = {QCH[q + 1] - 1: q for q in range(NSC)}

            for t in range(NCH):
                base = t * P
                nn = min(P, NS - base)
                xt = work.tile([P, FEAT], BF16, tag="xt")
                nc.sync.dma_start(out=xt[:nn, :], in_=x_sh[base:base + nn, :])
                dense_tile(xt, 0, t, base, nn, startup=True)
                if t in ag_after_chunk:
                    fire_ag(0, ag_after_chunk[t])

            def pair_spans(pair):
                ch0 = pair * PAIR_CH
                chn = min(PAIR_CH, NCH - ch0)
                spans = []
                for sc in range(NSC):
                    t0 = int(tile_off[ch0, sc])
                    ncall = int(sum(T[ch0 + ci, sc] for ci in range(chn)))
                    spans.append((t0, ncall))
                return ch0, chn, spans

            pend = {}
            # eqc load engine per sc: spread across gpsimd(SWDGE q0)/scalar/sync
            eqc_eng = [nc.sync, nc.sync, nc.scalar, nc.scalar]

            def issue_gather(layer, pair, sc, st):
                ch0, chn, spans = pair_spans(pair)
                t0, ncall = spans[sc]
                if ncall == 0:
                    return
                gout = gpool.tile([P, ncall, ROWB], BF16, tag="g")
                nc.gpsimd.dma_gather(
                    out_ap=gout[:],
                    in_ap=xwss_full[layer][sc][:],
                    idxs_ap=gidx_sb[:, t0 * 8:(t0 + ncall) * 8],
                    num_idxs=ncall * P, num_idxs_reg=ncall * P,
                    elem_size=ROWB, single_packet=False,
                    queue_num=(pair * NSC + sc) % 4)
                st["gouts"][sc] = gout

            def stage_loads(layer, pair):
                """issue merged eq loads + self-loop row loads for a pair"""
                ch0, chn, spans = pair_spans(pair)
                st = {"gouts": [None] * NSC, "ecs": [None] * NSC}
                for sc in range(NSC):
                    t0, ncall = spans[sc]
                    if ncall == 0:
                        continue
                    ec = eqp.tile([P, 2 * ncall, P], FP8, tag="ec")
                    eqc_eng[sc].dma_start(
                        out=ec[:], in_=eqc_t[:, 2 * t0 * P:2 * (t0 + ncall) * P])
                    st["ecs"][sc] = ec
                pend[(layer, pair)] = st

            def weight_feats(out_ap, in_bf, in_f8, exp_ap, tdim):
                """out cols 0:128 = feats * exp (per-head bcast); feats split
                bf16 0:120 + fp8 120:128 (original feature order restored)."""
                if tdim is None:
                    nc.vector.tensor_tensor(
                        out=out_ap[:, 0:96].rearrange("p (h c) -> p h c", h=3),
                        in0=in_bf[:, 0:96].rearrange("p (h c) -> p h c", h=3),
                        in1=exp_ap[:, 0:3].unsqueeze(2).broadcast_to([P, 3, 32]),
                        op=mybir.AluOpType.mult)
                    nc.vector.tensor_tensor(
                        out=out_ap[:, 96:120],
                        in0=in_bf[:, 96:120],
                        in1=exp_ap[:, 3:4].broadcast_to([P, 24]),
                        op=mybir.AluOpType.mult)
                    nc.vector.tensor_tensor(
                        out=out_ap[:, 120:128],
                        in0=in_f8,
                        in1=exp_ap[:, 3:4].broadcast_to([P, 8]),
                        op=mybir.AluOpType.mult)
                else:
                    t = tdim
                    nc.vector.tensor_tensor(
                        out=out_ap[:, :, 0:96].rearrange("p t (h c) -> p t h c", h=3),
                        in0=in_bf[:, :, 0:96].rearrange("p t (h c) -> p t h c", h=3),
                        in1=exp_ap[:, :, 0:3].unsqueeze(3).broadcast_to([P, t, 3, 32]),
                        op=mybir.AluOpType.mult)
                    nc.vector.tensor_tensor(
                        out=out_ap[:, :, 96:120],
                        in0=in_bf[:, :, 96:120],
                        in1=exp_ap[:, :, 3:4].broadcast_to([P, t, 24]),
                        op=mybir.AluOpType.mult)
                    nc.vector.tensor_tensor(
                        out=out_ap[:, :, 120:128],
                        in0=in_f8,
                        in1=exp_ap[:, :, 3:4].broadcast_to([P, t, 8]),
                        op=mybir.AluOpType.mult)

            def compute_pair(layer, pair):
                ch0, chn, spans = pair_spans(pair)
                st = pend.pop((layer, pair))

                acc_ps = [accp.tile([P, NCOL], F32, tag="acc",
                                    name=f"acc_l{layer}p{pair}c{ci}") for ci in range(chn)]
                mm_count = [0] * chn
                mm_total = [int(T[ch0 + ci, :].sum()) + 1 for ci in range(chn)]

                # self-loop tiles: local rows, diagonal scatter (first mm)
                for ci in range(chn):
                    sl = slcache[:, ch0 + ci, :]
                    alsl = work.tile([P, 4], F32, tag="alsl")
                    nc.vector.tensor_tensor(out=alsl[:], in0=sl[:, BFN:BFN + 4],
                                            in1=sdcache[:, ch0 + ci, :],
                                            op=mybir.AluOpType.add)
                    nc.scalar.activation(out=alsl[:], in_=alsl[:], func=AF.Prelu,
                                         alpha=SLOPE)
                    wsl = work.tile([P, MMC], BF16, tag="wsl")
                    nc.scalar.activation(out=wsl[:, 128:132], in_=alsl[:], func=AF.Exp)
                    weight_feats(wsl[:], sl[:], sl[:, BFN + 4:ROWB].bitcast(FP8),
                                 wsl[:, 128:132], None)
                    mm_count[ci] += 1
                    nc.tensor.matmul(out=acc_ps[ci][:, 0:MMC], lhsT=diag[:],
                                     rhs=wsl[:], start=True,
                                     stop=(mm_count[ci] == mm_total[ci]))

                for sc in range(NSC):
                    t0, ncall = spans[sc]
                    if ncall == 0:
                        continue
                    gout, ec = st["gouts"][sc], st["ecs"][sc]
                    sd_ps = sdp.tile([P, ncall, 4], F32, tag="sdps")
                    tlist = []
                    slot = 0
                    for ci in range(chn):
                        for _ in range(int(T[ch0 + ci, sc])):
                            nc.tensor.matmul(out=sd_ps[:, slot, :],
                                             lhsT=ec[:, slot, :],
                                             rhs=sdcache[:, ch0 + ci, :],
                                             start=True, stop=True)
                            tlist.append(ci)
                            slot += 1

                    al = work.tile([P, ncall, 4], F32, tag="al")
                    nc.vector.tensor_tensor(out=al[:], in0=gout[:, :, BFN:BFN + 4],
                                            in1=sd_ps[:], op=mybir.AluOpType.add)
                    nc.scalar.activation(out=al[:], in_=al[:], func=AF.Prelu,
                                         alpha=SLOPE)
                    wrhs = wrhsp.tile([P, ncall, MMC], BF16, tag="wr")
                    nc.scalar.activation(out=wrhs[:, :, 128:132], in_=al[:],
                                         func=AF.Exp)
                    weight_feats(wrhs[:], gout[:],
                                 gout[:, :, BFN + 4:ROWB].bitcast(FP8),
                                 wrhs[:, :, 128:132], ncall)

                    for slot, ci in enumerate(tlist):
                        mm_count[ci] += 1
                        nc.tensor.matmul(
                            out=acc_ps[ci][:, 0:MMC], lhsT=ec[:, ncall + slot, :],
                            rhs=wrhs[:, slot, :],
                            start=False,
                            stop=(mm_count[ci] == mm_total[ci]))

                # batched reciprocal of softmax denominators
                dnm = dnmp.tile([P, chn, 4], F32, tag="dnm")
                for ci in range(chn):
                    nc.scalar.copy(out=dnm[:, ci, :], in_=acc_ps[ci][:, 128:132])
                rcp = dnmp.tile([P, chn, 4], F32, tag="rcp")
                nc.vector.reciprocal(out=rcp[:], in_=dnm[:])

                for ci in range(chn):
                    base = (ch0 + ci) * P
                    nn = min(P, NS - base)
                    if layer < 2:
                        h = nodep.tile([P, FEAT], BF16, tag="h")
                        nc.vector.tensor_tensor(
                            out=h[:].rearrange("p (h c) -> p h c", h=4),
                            in0=acc_ps[ci][:, 0:128].rearrange("p (h c) -> p h c", h=4),
                            in1=rcp[:, ci, :].unsqueeze(2).broadcast_to([P, 4, 32]),
                            op=mybir.AluOpType.mult)
                        nc.vector.tensor_tensor(out=h[:], in0=h[:], in1=btile[layer][:],
                                                op=mybir.AluOpType.add)
                        mn = nodep.tile([P, FEAT], BF16, tag="mn")
                        nc.vector.tensor_scalar(out=mn[:], in0=h[:], scalar1=0.0,
                                                scalar2=None, op0=mybir.AluOpType.min)
                        nc.scalar.activation(out=mn[:], in_=mn[:],
                                             func=AF.Exp)
                        nc.vector.tensor_scalar(out=h[:], in0=h[:], scalar1=0.0,
                                                scalar2=-1.0, op0=mybir.AluOpType.max,
                                                op1=mybir.AluOpType.add)
                        nc.vector.tensor_tensor(out=h[:], in0=h[:], in1=mn[:],
                                                op=mybir.AluOpType.add)
                        dense_tile(h, layer + 1, ch0 + ci, base, nn)
                        if (ch0 + ci) in ag_after_chunk:
                            fire_ag(layer + 1, ag_after_chunk[ch0 + ci])
                    else:
                        hf = nodep.tile([P, FEAT], F32, tag="hf")
                        nc.vector.tensor_tensor(
                            out=hf[:].rearrange("p (h c) -> p h c", h=4),
                            in0=acc_ps[ci][:, 0:128].rearrange("p (h c) -> p h c", h=4),
                            in1=rcp[:, ci, :].unsqueeze(2).broadcast_to([P, 4, 32]),
                            op=mybir.AluOpType.mult)
                        o = nodep.tile([P, OUT], F32, tag="o")
                        hv = hf[:].rearrange("p (h c) -> p h c", h=4)
                        nc.vector.tensor_tensor(out=o[:], in0=hv[:, 0, :], in1=hv[:, 1, :],
                                                op=mybir.AluOpType.add)
                        nc.vector.tensor_tensor(out=o[:], in0=o[:], in1=hv[:, 2, :],
                                                op=mybir.AluOpType.add)
                        nc.vector.tensor_tensor(out=o[:], in0=o[:], in1=hv[:, 3, :],
                                                op=mybir.AluOpType.add)
                        nc.vector.tensor_scalar(out=o[:], in0=o[:], scalar1=0.25,
                                                scalar2=None, op0=mybir.AluOpType.mult)
                        nc.vector.tensor_tensor(out=o[:], in0=o[:], in1=btile[2][:],
                                                op=mybir.AluOpType.add)
                        nc.sync.dma_start(out=out_t[base:base + nn, :], in_=o[:nn, :])

            # layer-0 warmup: all AG triggers are already queued (dense loop
            # above); issue pairs 0-1 gathers quarter-major so each gather
            # unblocks right as its quarter's AllGather lands.
            stage_loads(0, 0)
            stage_loads(0, 1)
            for sc in range(NSC):
                issue_gather(0, 0, sc, pend[(0, 0)])
                issue_gather(0, 1, sc, pend[(0, 1)])

            for layer in range(3):
                for pair in range(NPAIR):
                    nxt = pair + 1
                    if layer == 0 and nxt < 2:
                        pass  # staged in warmup
                    elif nxt < NPAIR:
                        stage_loads(layer, nxt)
                        for sc in range(3):
                            issue_gather(layer, nxt, sc, pend[(layer, nxt)])
                    elif layer + 1 < 3:
                        stage_loads(layer + 1, 0)
                        for sc in range(3):
                            issue_gather(layer + 1, 0, sc, pend[(layer + 1, 0)])
                    if pend[(layer, pair)]["gouts"][3] is None:
                        issue_gather(layer, pair, 3, pend[(layer, pair)])
                    compute_pair(layer, pair)
    nc.compile()
    return nc


def kernel(x, edge_index, W1, as1, ad1, b1, W2, as2, ad2, b2, W3, as3, ad3, b3):
    x = np.asarray(x, np.float32)
    edge_index = np.asarray(edge_index)
    T, tile_off, ntiles, gidx, dstloc = _prep_graph(edge_index)
    nc = _build_program(T, tile_off, ntiles)

    wext = [_build_wext(np.asarray(W1, np.float32), np.asarray(as1, np.float32), np.asarray(ad1, np.float32)),
            _build_wext(np.asarray(W2, np.float32), np.asarray(as2, np.float32), np.asarray(ad2, np.float32)),
            _build_wext(np.asarray(W3, np.float32), np.asarray(as3, np.float32), np.asarray(ad3, np.float32))]
    bt = [np.ascontiguousarray(np.tile(np.asarray(b, np.float32)[None, :], (P, 1)))
          for b in (b1, b2, b3)]
    ident_np = np.eye(P, dtype=NPBF)
    diag_np = np.eye(P, dtype=NPF8)
    jj = np.arange(P, dtype=np.float32)

    # merged [eqT | eq] blocks, contiguous per (pair, sc) group
    NPAIR = (NCH + PAIR_CH - 1) // PAIR_CH
    group_spans = []
    for pair in range(NPAIR):
        ch0 = pair * PAIR_CH
        chn = min(PAIR_CH, NCH - ch0)
        for sc in range(NSC):
            t0 = int(tile_off[ch0, sc])
            ncall = int(sum(T[ch0 + ci, sc] for ci in range(chn)))
            group_spans.append((t0, ncall))

    in_maps = []
    for c in range(NCORES):
        dl = dstloc[c]
        eq_full = (dl[:, :, None] == jj[None, None, :])
        eq_np = np.ascontiguousarray(
            eq_full.transpose(1, 0, 2).reshape(P, ntiles * P)).astype(NPF8)
        eqT_np = np.ascontiguousarray(
            eq_full.transpose(2, 0, 1).reshape(P, ntiles * P)).astype(NPF8)
        eqc_np = np.zeros((P, ntiles * 2 * P), dtype=NPF8)
        for (t0, ncall) in group_spans:
            if ncall == 0:
                continue
            b0 = 2 * t0 * P
            eqc_np[:, b0:b0 + ncall * P] = eqT_np[:, t0 * P:(t0 + ncall) * P]
            eqc_np[:, b0 + ncall * P:b0 + 2 * ncall * P] = eq_np[:, t0 * P:(t0 + ncall) * P]
        m = {
            "x_shard": x[c * NS:(c + 1) * NS].astype(NPBF),
            "gidx": _wrap_idx(gidx[c]),
            "eqc": eqc_np,
            "ident": ident_np,
            "diag": diag_np,
        }
        for l in range(3):
            m[f"wext{l}"] = wext[l].astype(NPBF)
            m[f"btile{l}"] = bt[l]
        in_maps.append(m)

    trace = bool(int(os.environ.get("GAT_TRACE", "0")))
    res = run_bass_kernel_spmd(nc, in_maps, list(range(NCORES)), trace=trace)
    kernel.last_exec_time_ns = res.exec_time_ns
    out = np.concatenate([res.results[c]["out"] for c in range(NCORES)], axis=0)
    return out


kernel.last_exec_time_ns = None


# revision 26
# speedup vs baseline: 1.0340x; 1.0340x over previous
"""Distributed 3-layer GAT on 8 TRN2 NeuronCores (Bass/Tile).

Sharding: core c owns dst nodes [c*NS, (c+1)*NS). Non-self-loop edges
partitioned by dst shard, sorted by (src-quarter, dst-chunk, dst);
quarter-major tile numbering. Weights replicated.

Gather table rows are 256 B: 120 bf16 features + 4 bf16 s_src scores +
8 fp8e4m3 features (wext columns permuted host-side so the dense matmul
emits them contiguously; error budget verified offline: rel ~1.4e-2).
s_dst lives in a separate tiny [NS,4] bf16 table consumed by the eqT
matmuls. Halving the row (512->256 B) halves both the SWDGE gather
stream and the AllGather volume.

Per layer: sharded dense (bf16) -> 4 chunked AllGathers -> edge phase:
dma_gather of src rows round-robined over SWDGE queues 1-3 (sc3 issued
one pair late so the AG3 wait never head-of-line blocks the gpsimd
queue), merged [eqT|eq] fp8 one-hot loads spread over gpsimd-q0 /
scalar / sync DMA queues, PE matmuls accumulate weighted messages +
softmax denominators in PSUM. Self-loops skip the gather entirely:
local rows + constant diagonal fp8 lhsT. alpha = ss + (eqT @ sd) in
f32, leaky-relu via Act Prelu, exp lands in mm-rhs cols 128:132 and
doubles as the broadcast weighting multiplier. Reciprocals batched per
pair.
"""
import os, sys, types
sys.path.insert(0, "/opt/trn_rl_repo")
import numpy as np
import ml_dtypes

import antenv
if "antenv.axon_hooks" not in sys.modules:
    _hooks_mod = types.ModuleType("antenv.axon_hooks")
    _HOOK = [None]
    _hooks_mod.set_axon_ntff_profile_hook = lambda h: _HOOK.__setitem__(0, h)
    _hooks_mod.get_axon_ntff_profile_hook = lambda: _HOOK[0]
    sys.modules["antenv.axon_hooks"] = _hooks_mod
    antenv.axon_hooks = _hooks_mod
    try:
        import trn_agent_boot.trn_boot as _tb
        _h = _tb._ntff_profile_via_ctypes("/opt/axon/libaxon_pjrt.so")
        if _h is not None:
            _hooks_mod.set_axon_ntff_profile_hook(_h)
    except Exception:
        pass

from concourse import bass, bacc, mybir, tile
from concourse import bass_utils
from concourse.bass_utils import run_bass_kernel_spmd
from concourse._compat import get_trn_type
bass_utils.upload_artifacts = lambda tmpdir: "local://noop"

F32 = mybir.dt.float32
BF16 = mybir.dt.bfloat16
FP8 = mybir.dt.float8e4
I16 = mybir.dt.int16
NPBF = ml_dtypes.bfloat16
NPF8 = ml_dtypes.float8_e4m3fn
P = 128
SLOPE = 0.2
AF = mybir.ActivationFunctionType

N = 100000
IN, HID, HEADS, OUT = 128, 32, 4, 32
NCORES = 8
NS = N // NCORES
NCH = (NS + P - 1) // P
PAIR_CH = 5
NSC = 4
ROWB = 128   # table row: 128 bf16 slots = 256 B
FEAT = 128
BFN = 120    # features stored in bf16 (row cols 0:120)
NCOL = 136   # dense output cols: 120 bf | 4 ws | 8 f8-feat | 4 wd
MMC = 132    # acc matmul rhs cols: 128 weighted feats + 4 exp

# quarter-major table layout: quarter q = per-core chunks [QCH[q], QCH[q+1])
QCH = [0, 30, 60, 90, 98]
S_ROW = [0, 3840, 7680, 11520]          # per-core row offset of quarter q
SZ = [3840, 3840, 3840, 980]            # per-core rows in quarter q
B_ROW = [0, 30720, 61440, 92160]        # global block start of quarter q
# AG for quarter q fires after the dense tile of chunk QCH[q+1]-1


def _build_wext(W, a_s, a_d):
    Fin = W.shape[0]
    H, C = a_s.shape
    Wr = W.reshape(Fin, H, C)
    ws = np.einsum("fhc,hc->fh", Wr, a_s)
    wd = np.einsum("fhc,hc->fh", Wr, a_d)
    # permuted: [feat0:120 | ws | feat120:128 | wd]
    return np.ascontiguousarray(
        np.concatenate([W[:, 0:BFN], ws, W[:, BFN:FEAT], wd], axis=1), np.float32)


def _perm_rows(v):
    """global node id -> permuted (quarter-major) table row"""
    c = v // NS
    l = v % NS
    q = np.searchsorted(S_ROW, l, side="right") - 1
    szq = np.asarray(SZ, np.int64)[q]
    return (np.asarray(B_ROW, np.int64)[q] + c * szq
            + (l - np.asarray(S_ROW, np.int64)[q])), q


def _prep_graph(edge_index):
    # self-loops are added separately (local rows, no gather); all original
    # edges stay (PyG adds loops on top of whatever is in edge_index).
    src = edge_index[0].astype(np.int64)
    dst = edge_index[1].astype(np.int64)
    prow_all, q_all = _perm_rows(src)

    core = dst // NS
    per_core = []
    counts = np.zeros((NCORES, NCH, NSC), dtype=np.int64)
    for c in range(NCORES):
        m = core == c
        s_c, d_c = prow_all[m], dst[m]
        sc = q_all[m]
        dl = d_c - c * NS
        ch = dl // P
        order = np.lexsort((d_c, ch, sc))
        s_c, dl, ch, sc = s_c[order], dl[order], ch[order], sc[order]
        np.add.at(counts[c], (ch, sc), 1)
        per_core.append((s_c, dl, ch, sc))

    T = np.ceil(counts.max(axis=0) / P).astype(np.int64)
    tile_off = np.zeros((NCH, NSC), dtype=np.int64)
    acc = 0
    for sc in range(NSC):
        for ch in range(NCH):
            tile_off[ch, sc] = acc
            acc += T[ch, sc]
    ntiles = acc

    gidx = np.zeros((NCORES, ntiles * P), dtype=np.int16)
    dstloc = np.full((NCORES, ntiles, P), -1.0, dtype=np.float32)
    for c in range(NCORES):
        s_c, dl, ch, sc = per_core[c]
        pos = 0
        for scv in range(NSC):
            for chv in range(NCH):
                n = int(counts[c, chv, scv])
                if n == 0:
                    continue
                t0 = int(tile_off[chv, scv])
                gidx[c, t0 * P: t0 * P + n] = (s_c[pos:pos + n] - B_ROW[scv]).astype(np.int16)
                dstloc[c].reshape(-1)[t0 * P: t0 * P + n] = (dl[pos:pos + n] - chv * P).astype(np.float32)
                pos += n
        assert pos == len(s_c)
    return T, tile_off, int(ntiles), gidx, dstloc


def _wrap_idx(flat16):
    n = flat16.shape[0]
    w = flat16.reshape(n // 16, 16).T
    return np.ascontiguousarray(np.tile(w, (8, 1)), np.int16)


def _build_program(T, tile_off, ntiles):
    nc = bacc.Bacc(get_trn_type() or "TRN2", target_bir_lowering=False,
                   debug=False, enable_asserts=False, num_devices=NCORES,
                   num_swdge_queues=4)
    x_sh = nc.dram_tensor("x_shard", [NS, FEAT], BF16, kind="ExternalInput").ap()
    gidx_t = nc.dram_tensor("gidx", [P, ntiles * 8], I16, kind="ExternalInput").ap()
    eqc_t = nc.dram_tensor("eqc", [P, ntiles * 2 * P], FP8, kind="ExternalInput").ap()
    wext_t = [nc.dram_tensor(f"wext{l}", [FEAT, NCOL], BF16, kind="ExternalInput").ap() for l in range(3)]
    btile_t = [nc.dram_tensor(f"btile{l}", [P, FEAT if l < 2 else OUT], F32, kind="ExternalInput").ap() for l in range(3)]
    ident_t = nc.dram_tensor("ident", [P, P], BF16, kind="ExternalInput").ap()
    diag_t = nc.dram_tensor("diag", [P, P], FP8, kind="ExternalInput").ap()
    out_t = nc.dram_tensor("out", [NS, OUT], F32, kind="ExternalOutput").ap()

    groups = [list(range(NCORES))]
    NPAIR = (NCH + PAIR_CH - 1) // PAIR_CH

    with tile.TileContext(nc) as tc:
        with (
            tc.tile_pool(name="const", bufs=1) as constp,
            tc.tile_pool(name="sched", bufs=1) as schedp,
            tc.tile_pool(name="gpool", bufs=9) as gpool,
            tc.tile_pool(name="eqp", bufs=9) as eqp,
            tc.tile_pool(name="work", bufs=5) as work,
            tc.tile_pool(name="wrhsp", bufs=3) as wrhsp,
            tc.tile_pool(name="nodep", bufs=6) as nodep,
            tc.tile_pool(name="dnmp", bufs=2) as dnmp,
            tc.tile_pool(name="accp", bufs=5, space="PSUM") as accp,
            tc.tile_pool(name="tpp", bufs=1, space="PSUM") as tpp,
            tc.tile_pool(name="sdp", bufs=2, space="PSUM") as sdp,
            tc.tile_pool(name="dram", bufs=2, space="DRAM") as dramp,
        ):
            ident = constp.tile([P, P], BF16, tag="ident")
            nc.sync.dma_start(out=ident[:], in_=ident_t[:])
            diag = constp.tile([P, P], FP8, tag="diag")
            nc.sync.dma_start(out=diag[:], in_=diag_t[:])
            wext, btile = [], []
            for l in range(3):
                w = constp.tile([FEAT, NCOL], BF16, tag=f"wext{l}")
                nc.sync.dma_start(out=w[:], in_=wext_t[l][:])
                wext.append(w)
                b = constp.tile([P, FEAT if l < 2 else OUT], F32, tag=f"bt{l}")
                nc.sync.dma_start(out=b[:], in_=btile_t[l][:])
                btile.append(b)
            gidx_sb = schedp.tile([P, ntiles * 8], I16, tag="gidx")
            nc.scalar.dma_start(out=gidx_sb[:], in_=gidx_t[:])

            xwss_sh = [dramp.tile([NS, ROWB], BF16, tag="xwsh", name=f"xwsh{i}") for i in range(3)]
            sdcache = schedp.tile([P, NCH, 4], BF16, tag="sdc")
            slcache = schedp.tile([P, NCH, ROWB], BF16, tag="slc")
            xwss_full = [[dramp.tile([NCORES * SZ[q], ROWB], BF16, tag=f"xwfull{q}",
                                     name=f"xwfull{i}_{q}", addr_space="Shared")
                          for q in range(NSC)] for i in range(3)]

            def fire_ag(layer, q):
                nc.gpsimd.collective_compute(
                    "AllGather", mybir.AluOpType.bypass, replica_groups=groups,
                    ins=[xwss_sh[layer][S_ROW[q]:S_ROW[q] + SZ[q], :].opt()],
                    outs=[xwss_full[layer][q][:].opt()])

            def dense_tile(h_sb, lnext, ch, base, nn, startup=False):
                hT_ps = tpp.tile([P, P], BF16, tag="tp")
                nc.tensor.transpose(out=hT_ps[:], in_=h_sb[:], identity=ident[:])
                hT_sb = work.tile([P, P], BF16, tag="hT")
                nc.scalar.copy(out=hT_sb[:], in_=hT_ps[:])
                if startup:
                    d_ps = accp.tile([P, NCOL], F32, tag="acc", name="d_ps_s")
                else:
                    d_ps = tpp.tile([P, NCOL], F32, tag="tp", name="d_ps")
                nc.tensor.matmul(out=d_ps[:], lhsT=hT_sb[:], rhs=wext[lnext][:],
                                 start=True, stop=True)
                nc.scalar.copy(out=slcache[:, ch, 0:BFN + 4], in_=d_ps[:, 0:BFN + 4])
                nc.scalar.copy(out=slcache[:, ch, BFN + 4:ROWB].bitcast(FP8),
                               in_=d_ps[:, BFN + 4:BFN + 12])
                nc.scalar.copy(out=sdcache[:, ch, :], in_=d_ps[:, BFN + 12:NCOL])
                nc.scalar.dma_start(out=xwss_sh[lnext][base:base + nn, :],
                                    in_=slcache[:nn, ch, :])

            ag_after_chunk = {QCH[q + 1] - 1: q for q in range(NSC)}

            for t in range(NCH):
                base = t * P
                nn = min(P, NS - base)
                xt = work.tile([P, FEAT], BF16, tag="xt")
                nc.sync.dma_start(out=xt[:nn, :], in_=x_sh[base:base + nn, :])
                dense_tile(xt, 0, t, base, nn, startup=True)
                if t in ag_after_chunk:
                    fire_ag(0, ag_after_chunk[t])

            def pair_spans(pair):
                ch0 = pair * PAIR_CH
                chn = min(PAIR_CH, NCH - ch0)
                spans = []
                for sc in range(NSC):
                    t0 = int(tile_off[ch0, sc])
                    ncall = int(sum(T[ch0 + ci, sc] for ci in range(chn)))
                    spans.append((t0, ncall))
                return ch0, chn, spans

            pend = {}
            # eqc load engine per sc: spread across gpsimd(SWDGE q0)/scalar/sync
            eqc_eng = [nc.sync, nc.sync, nc.scalar, nc.scalar]

            def issue_gather(layer, pair, sc, st):
                ch0, chn, spans = pair_spans(pair)
                t0, ncall = spans[sc]
                if ncall == 0:
                    return
                gout = gpool.tile([P, ncall, ROWB], BF16, tag="g")
                nc.gpsimd.dma_gather(
                    out_ap=gout[:],
                    in_ap=xwss_full[layer][sc][:],
                    idxs_ap=gidx_sb[:, t0 * 8:(t0 + ncall) * 8],
                    num_idxs=ncall * P, num_idxs_reg=ncall * P,
                    elem_size=ROWB, single_packet=False,
                    queue_num=(pair * NSC + sc) % 4)
                st["gouts"][sc] = gout

            def stage_loads(layer, pair):
                """issue merged eq loads + self-loop row loads for a pair"""
                ch0, chn, spans = pair_spans(pair)
                st = {"gouts": [None] * NSC, "ecs": [None] * NSC}
                for sc in range(NSC):
                    t0, ncall = spans[sc]
                    if ncall == 0:
                        continue
                    ec = eqp.tile([P, 2 * ncall, P], FP8, tag="ec")
                    eqc_eng[sc].dma_start(
                        out=ec[:], in_=eqc_t[:, 2 * t0 * P:2 * (t0 + ncall) * P])
                    st["ecs"][sc] = ec
                pend[(layer, pair)] = st

            def weight_feats(out_ap, in_bf, in_f8, exp_ap, tdim):
                """out cols 0:128 = feats * exp (per-head bcast); feats split
                bf16 0:120 + fp8 120:128 (original feature order restored)."""
                if tdim is None:
                    nc.vector.tensor_tensor(
                        out=out_ap[:, 0:96].rearrange("p (h c) -> p h c", h=3),
                        in0=in_bf[:, 0:96].rearrange("p (h c) -> p h c", h=3),
                        in1=exp_ap[:, 0:3].unsqueeze(2).broadcast_to([P, 3, 32]),
                        op=mybir.AluOpType.mult)
                    nc.vector.tensor_tensor(
                        out=out_ap[:, 96:120],
                        in0=in_bf[:, 96:120],
                        in1=exp_ap[:, 3:4].broadcast_to([P, 24]),
                        op=mybir.AluOpType.mult)
                    nc.vector.tensor_tensor(
                        out=out_ap[:, 120:128],
                        in0=in_f8,
                        in1=exp_ap[:, 3:4].broadcast_to([P, 8]),
                        op=mybir.AluOpType.mult)
                else:
                    t = tdim
                    nc.vector.tensor_tensor(
                        out=out_ap[:, :, 0:96].rearrange("p t (h c) -> p t h c", h=3),
                        in0=in_bf[:, :, 0:96].rearrange("p t (h c) -> p t h c", h=3),
                        in1=exp_ap[:, :, 0:3].unsqueeze(3).broadcast_to([P, t, 3, 32]),
                        op=mybir.AluOpType.mult)
                    nc.vector.tensor_tensor(
                        out=out_ap[:, :, 96:120],
                        in0=in_bf[:, :, 96:120],
                        in1=exp_ap[:, :, 3:4].broadcast_to([P, t, 24]),
                        op=mybir.AluOpType.mult)
                    nc.vector.tensor_tensor(
                        out=out_ap[:, :, 120:128],
                        in0=in_f8,
                        in1=exp_ap[:, :, 3:4].broadcast_to([P, t, 8]),
                        op=mybir.AluOpType.mult)

            def compute_pair(layer, pair):
                ch0, chn, spans = pair_spans(pair)
                st = pend.pop((layer, pair))

                acc_ps = [accp.tile([P, NCOL], F32, tag="acc",
                                    name=f"acc_l{layer}p{pair}c{ci}") for ci in range(chn)]
                mm_count = [0] * chn
                mm_total = [int(T[ch0 + ci, :].sum()) + 1 for ci in range(chn)]

                # self-loop tiles: local rows, diagonal scatter (first mm)
                for ci in range(chn):
                    sl = slcache[:, ch0 + ci, :]
                    alsl = work.tile([P, 4], F32, tag="alsl")
                    nc.vector.tensor_tensor(out=alsl[:], in0=sl[:, BFN:BFN + 4],
                                            in1=sdcache[:, ch0 + ci, :],
                                            op=mybir.AluOpType.add)
                    nc.scalar.activation(out=alsl[:], in_=alsl[:], func=AF.Prelu,
                                         alpha=SLOPE)
                    wsl = work.tile([P, MMC], BF16, tag="wsl")
                    nc.scalar.activation(out=wsl[:, 128:132], in_=alsl[:], func=AF.Exp)
                    weight_feats(wsl[:], sl[:], sl[:, BFN + 4:ROWB].bitcast(FP8),
                                 wsl[:, 128:132], None)
                    mm_count[ci] += 1
                    nc.tensor.matmul(out=acc_ps[ci][:, 0:MMC], lhsT=diag[:],
                                     rhs=wsl[:], start=True,
                                     stop=(mm_count[ci] == mm_total[ci]))

                for sc in range(NSC):
                    t0, ncall = spans[sc]
                    if ncall == 0:
                        continue
                    gout, ec = st["gouts"][sc], st["ecs"][sc]
                    sd_ps = sdp.tile([P, ncall, 4], F32, tag="sdps")
                    tlist = []
                    slot = 0
                    for ci in range(chn):
                        for _ in range(int(T[ch0 + ci, sc])):
                            nc.tensor.matmul(out=sd_ps[:, slot, :],
                                             lhsT=ec[:, slot, :],
                                             rhs=sdcache[:, ch0 + ci, :],
                                             start=True, stop=True)
                            tlist.append(ci)
                            slot += 1

                    al = work.tile([P, ncall, 4], F32, tag="al")
                    nc.vector.tensor_tensor(out=al[:], in0=gout[:, :, BFN:BFN + 4],
                                            in1=sd_ps[:], op=mybir.AluOpType.add)
                    nc.scalar.activation(out=al[:], in_=al[:], func=AF.Prelu,
                                         alpha=SLOPE)
                    wrhs = wrhsp.tile([P, ncall, MMC], BF16, tag="wr")
                    nc.scalar.activation(out=wrhs[:, :, 128:132], in_=al[:],
                                         func=AF.Exp)
                    weight_feats(wrhs[:], gout[:],
                                 gout[:, :, BFN + 4:ROWB].bitcast(FP8),
                                 wrhs[:, :, 128:132], ncall)

                    for slot, ci in enumerate(tlist):
                        mm_count[ci] += 1
                        nc.tensor.matmul(
                            out=acc_ps[ci][:, 0:MMC], lhsT=ec[:, ncall + slot, :],
                            rhs=wrhs[:, slot, :],
                            start=False,
                            stop=(mm_count[ci] == mm_total[ci]))

                # batched reciprocal of softmax denominators
                dnm = dnmp.tile([P, chn, 4], F32, tag="dnm")
                for ci in range(chn):
                    nc.scalar.copy(out=dnm[:, ci, :], in_=acc_ps[ci][:, 128:132])
                rcp = dnmp.tile([P, chn, 4], F32, tag="rcp")
                nc.vector.reciprocal(out=rcp[:], in_=dnm[:])

                for ci in range(chn):
                    base = (ch0 + ci) * P
                    nn = min(P, NS - base)
                    if layer < 2:
                        h = nodep.tile([P, FEAT], BF16, tag="h")
                        nc.vector.tensor_tensor(
                            out=h[:].rearrange("p (h c) -> p h c", h=4),
                            in0=acc_ps[ci][:, 0:128].rearrange("p (h c) -> p h c", h=4),
                            in1=rcp[:, ci, :].unsqueeze(2).broadcast_to([P, 4, 32]),
                            op=mybir.AluOpType.mult)
                        nc.vector.tensor_tensor(out=h[:], in0=h[:], in1=btile[layer][:],
                                                op=mybir.AluOpType.add)
                        mn = nodep.tile([P, FEAT], BF16, tag="mn")
                        nc.vector.tensor_scalar(out=mn[:], in0=h[:], scalar1=0.0,
                                                scalar2=None, op0=mybir.AluOpType.min)
                        nc.scalar.activation(out=mn[:], in_=mn[:],
                                             func=AF.Exp)
                        nc.vector.tensor_scalar(out=h[:], in0=h[:], scalar1=0.0,
                                                scalar2=-1.0, op0=mybir.AluOpType.max,
                                                op1=mybir.AluOpType.add)
                        nc.vector.tensor_tensor(out=h[:], in0=h[:], in1=mn[:],
                                                op=mybir.AluOpType.add)
                        dense_tile(h, layer + 1, ch0 + ci, base, nn)
                        if (ch0 + ci) in ag_after_chunk:
                            fire_ag(layer + 1, ag_after_chunk[ch0 + ci])
                    else:
                        hf = nodep.tile([P, FEAT], F32, tag="hf")
                        nc.vector.tensor_tensor(
                            out=hf[:].rearrange("p (h c) -> p h c", h=4),
                            in0=acc_ps[ci][:, 0:128].rearrange("p (h c) -> p h c", h=4),
                            in1=rcp[:, ci, :].unsqueeze(2).broadcast_to([P, 4, 32]),
                            op=mybir.AluOpType.mult)
                        o = nodep.tile([P, OUT], F32, tag="o")
                        hv = hf[:].rearrange("p (h c) -> p h c", h=4)
                        nc.vector.tensor_tensor(out=o[:], in0=hv[:, 0, :], in1=hv[:, 1, :],
                                                op=mybir.AluOpType.add)
                        nc.vector.tensor_tensor(out=o[:], in0=o[:], in1=hv[:, 2, :],
                                                op=mybir.AluOpType.add)
                        nc.vector.tensor_tensor(out=o[:], in0=o[:], in1=hv[:, 3, :],
                                                op=mybir.AluOpType.add)
                        nc.vector.tensor_scalar(out=o[:], in0=o[:], scalar1=0.25,
                                                scalar2=None, op0=mybir.AluOpType.mult)
                        nc.vector.tensor_tensor(out=o[:], in0=o[:], in1=btile[2][:],
                                                op=mybir.AluOpType.add)
                        nc.sync.dma_start(out=out_t[base:base + nn, :], in_=o[:nn, :])

            # layer-0 warmup: all AG triggers are already queued (dense loop
            # above); issue pairs 0-1 gathers quarter-major so each gather
            # unblocks right as its quarter's AllGather lands.
            stage_loads(0, 0)
            stage_loads(0, 1)
            for sc in range(NSC):
                issue_gather(0, 0, sc, pend[(0, 0)])
                issue_gather(0, 1, sc, pend[(0, 1)])

            for layer in range(3):
                for pair in range(NPAIR):
                    nxt = pair + 1
                    if layer == 0 and nxt < 2:
                        pass  # staged in warmup
                    elif nxt < NPAIR:
                        stage_loads(layer, nxt)
                        for sc in range(3):
                            issue_gather(layer, nxt, sc, pend[(layer, nxt)])
                    elif layer + 1 < 3:
                        stage_loads(layer + 1, 0)
                        for sc in range(3):
                            issue_gather(layer + 1, 0, sc, pend[(layer + 1, 0)])
                    if pend[(layer, pair)]["gouts"][3] is None:
                        issue_gather(layer, pair, 3, pend[(layer, pair)])
                    compute_pair(layer, pair)
    nc.compile()
    return nc


def kernel(x, edge_index, W1, as1, ad1, b1, W2, as2, ad2, b2, W3, as3, ad3, b3):
    x = np.asarray(x, np.float32)
    edge_index = np.asarray(edge_index)
    T, tile_off, ntiles, gidx, dstloc = _prep_graph(edge_index)
    nc = _build_program(T, tile_off, ntiles)

    wext = [_build_wext(np.asarray(W1, np.float32), np.asarray(as1, np.float32), np.asarray(ad1, np.float32)),
            _build_wext(np.asarray(W2, np.float32), np.asarray(as2, np.float32), np.asarray(ad2, np.float32)),
            _build_wext(np.asarray(W3, np.float32), np.asarray(as3, np.float32), np.asarray(ad3, np.float32))]
    bt = [np.ascontiguousarray(np.tile(np.asarray(b, np.float32)[None, :], (P, 1)))
          for b in (b1, b2, b3)]
    ident_np = np.eye(P, dtype=NPBF)
    diag_np = np.eye(P, dtype=NPF8)
    jj = np.arange(P, dtype=np.float32)

    # merged [eqT | eq] blocks, contiguous per (pair, sc) group
    NPAIR = (NCH + PAIR_CH - 1) // PAIR_CH
    group_spans = []
    for pair in range(NPAIR):
        ch0 = pair * PAIR_CH
        chn = min(PAIR_CH, NCH - ch0)
        for sc in range(NSC):
            t0 = int(tile_off[ch0, sc])
            ncall = int(sum(T[ch0 + ci, sc] for ci in range(chn)))
            group_spans.append((t0, ncall))

    in_maps = []
    for c in range(NCORES):
        dl = dstloc[c]
        eq_full = (dl[:, :, None] == jj[None, None, :])
        eq_np = np.ascontiguousarray(
            eq_full.transpose(1, 0, 2).reshape(P, ntiles * P)).astype(NPF8)
        eqT_np = np.ascontiguousarray(
            eq_full.transpose(2, 0, 1).reshape(P, ntiles * P)).astype(NPF8)
        eqc_np = np.zeros((P, ntiles * 2 * P), dtype=NPF8)
        for (t0, ncall) in group_spans:
            if ncall == 0:
                continue
            b0 = 2 * t0 * P
            eqc_np[:, b0:b0 + ncall * P] = eqT_np[:, t0 * P:(t0 + ncall) * P]
            eqc_np[:, b0 + ncall * P:b0 + 2 * ncall * P] = eq_np[:, t0 * P:(t0 + ncall) * P]
        m = {
            "x_shard": x[c * NS:(c + 1) * NS].astype(NPBF),
            "gidx": _wrap_idx(gidx[c]),
            "eqc": eqc_np,
            "ident": ident_np,
            "diag": diag_np,
        }
        for l in range(3):
            m[f"wext{l}"] = wext[l].astype(NPBF)
            m[f"btile{l}"] = bt[l]
        in_maps.append(m)

    trace = bool(int(os.environ.get("GAT_TRACE", "0")))
    res = run_bass_kernel_spmd(nc, in_maps, list(range(NCORES)), trace=trace)
    kernel.last_exec_time_ns = res.exec_time_ns
    out = np.concatenate([res.results[c]["out"] for c in range(NCORES)], axis=0)
    return out


kernel.last_exec_time_ns = None
